# revision 1
# baseline (speedup 1.0000x reference)
import numpy as np
import jax
import jax.numpy as jnp

# Problem constants (hardcoded — kernel.py must be self-contained).
B, DIM, H, W = 2, 192, 256, 256
HEADS = 4
WS = 8          # attention window size
NDEV = 8        # NeuronCores
RBLK = H // 4   # 64 rows per shard: 2 batches x 4 row-blocks = 8 shards
CP = DIM // HEADS


def _shard_fn(xs, qkv_w, dw_w, proj_w, temperature):
    """Compute one shard: xs is [192, 66, 256] fp32 (64 rows + 1-row halo each side).

    Pure data parallel over (batch, H-slab); the 1-row halo feeds the 3x3
    depthwise conv so slab outputs match the unsharded computation exactly.
    8x8 attention windows never cross slab boundaries (64 % 8 == 0).
    """
    c = DIM
    # 1x1 qkv conv as matmul over channels
    qkv = jnp.einsum('chw,oc->ohw', xs, qkv_w)  # [576, 66, 256]
    # depthwise 3x3: VALID in H (halo supplies context), SAME in W
    qkv = jax.lax.conv_general_dilated(
        qkv[None], dw_w, window_strides=(1, 1),
        padding=((0, 0), (1, 1)),
        feature_group_count=3 * c,
        dimension_numbers=('NCHW', 'OIHW', 'NCHW'))[0]  # [576, 64, 256]
    q, k, v = jnp.split(qkv, 3, axis=0)

    Hn, Wn = RBLK // WS, W // WS  # 8, 32

    def to_win(t):
        t = t.reshape(HEADS, CP, Hn, WS, Wn, WS)
        t = t.transpose(2, 4, 0, 1, 3, 5)
        return t.reshape(Hn * Wn, HEADS, CP, WS * WS)

    q, k, v = to_win(q), to_win(k), to_win(v)

    def l2n(t):
        return t / jnp.maximum(jnp.linalg.norm(t, axis=-1, keepdims=True), 1e-12)

    q, k = l2n(q), l2n(k)

    attn = jnp.einsum('nhcd,nhed->nhce', q, k) * temperature
    attn = jax.nn.softmax(attn, axis=-1)
    out = jnp.einsum('nhce,nhed->nhcd', attn, v)

    out = out.reshape(Hn, Wn, HEADS, CP, WS, WS)
    out = out.transpose(2, 3, 0, 4, 1, 5).reshape(c, RBLK, W)
    return jnp.einsum('chw,oc->ohw', out, proj_w)


_pfn = jax.pmap(_shard_fn)


def _replicate(a):
    a = np.asarray(a)
    return np.broadcast_to(a, (NDEV,) + a.shape)


def kernel(x, qkv_w, dw_w, proj_w, temperature):
    x = np.asarray(x, dtype=np.float32)
    # zero-pad H by 1 on each side (SAME padding at global edges), then cut
    # 8 halo'd slabs: shard d -> batch d//4, rows (d%4)*64 .. +66
    xp = np.pad(x, ((0, 0), (0, 0), (1, 1), (0, 0)))
    shards = np.stack([
        xp[d // 4, :, (d % 4) * RBLK: (d % 4) * RBLK + RBLK + 2, :]
        for d in range(NDEV)
    ])
    out = _pfn(shards, _replicate(qkv_w), _replicate(dw_w),
               _replicate(proj_w), _replicate(temperature))
    out = np.asarray(out)  # [8, 192, 64, 256]
    out = out.reshape(B, 4, DIM, RBLK, W).transpose(0, 2, 1, 3, 4)
    return np.ascontiguousarray(out.reshape(B, DIM, H, W))
